# revision 29
# baseline (speedup 1.0000x reference)
"""AutoCorrelation layer kernel for 8 Trainium2 NeuronCores (v3, data-parallel).

Math note: the reference's rfft/irfft pair over the zero-padded head dim
computes a circular cross-correlation; its mean over all lags collapses
analytically to (sum_d q_proj) * (sum_d k_proj) per head:
  corr[b,l] = (1/(H*L)) * sum_h (q[b,l] @ WqS + bqS)_h * (k[b,l] @ WkS + bkS)_h
with WqS = Wq.reshape(D,H,DK).sum(-1).  Downstream (top-6, softmax, weighted
value gather, output projection) follows the reference directly.

Distribution: data-parallel preprocessing (core i handles batch i only:
corr row, top-6, softmax-weighted value aggregation -> agg[1,256]), an
AllGather of the tiny agg vector, then each core computes its own
32768-column shard of the (256, 262144) output projection.

Precision: the corr/top-k path is fp32 (top-k margins as small as 4e-4
relative make bf16/fp16 unsafe); Wp/v/Wv/agg/out are bf16 (4.2e-3 max rel
error, verified offline against the reference).  Top-6 membership is decided
by is_ge against the 6th-largest value; the compared values only pass
through bit-faithful copies and a x1.0 fp32 PE transpose whose few-ulp
perturbation is ~4 orders of magnitude below the smallest top-k gap.

The all-zero bias case (the reference's setup) compiles a variant that
skips bias loads/adds entirely; nonzero biases get the general variant.
"""
import sys

sys.path.insert(0, "/opt/trn_rl_repo")

import ml_dtypes
import numpy as np

import concourse.bass as bass
import concourse.mybir as mybir
import concourse.tile as tile
from concourse import bacc
from concourse.bass_utils import run_bass_kernel_spmd
from concourse.masks import make_identity

F32 = mybir.dt.float32
BF16 = mybir.dt.bfloat16
I16 = mybir.dt.int16
NPBF16 = ml_dtypes.bfloat16

N_CORES = 8
B, L, D, H, DK = 8, 1024, 256, 8, 32
K_TOP = 6
NSH = (L * D) // N_CORES          # 32768 output cols per core
TILE_N = 2048
N_TILES = NSH // TILE_N           # 16
SUBS = TILE_N // 512              # 4
SCALE = 1.0 / (H * L)
WP_BUFS = 14

TRACE = False          # test harness sets this for profiled runs
LAST_RESULT = None     # stashed BassKernelResults from the last kernel() call
# "rep"  = every core redundantly preprocesses all 8 batches (no cross-core
#          traffic at all -> deterministic runtime);
# "cc"   = data-parallel preproc + runtime AllGather of agg (lower DMA but
#          the collective stack adds 10-100us of run-to-run jitter);
# "rdma" = data-parallel + raw peer DMA exchange (blocked by the Tile
#          scheduler's single-core deadlock check; kept for reference).
MODE = "rep"

_CACHE = {}


def _build_nc_rep():
    """All-batches-replicated variant (zero biases): no cross-core traffic."""
    nc = bacc.Bacc("TRN2", target_bir_lowering=False, debug=False, num_devices=N_CORES)

    # qk[p, b, c, l] = x[b][l, 128c+p]; batches 0-3 on sync, 4-7 on gpsimd
    qts = []
    kts = []
    for g in range(2):
        qts.append(nc.dram_tensor(f"qt{g}", [128, 4 * 2 * L], I16, kind="ExternalInput").ap())
        kts.append(nc.dram_tensor(f"kt{g}", [128, 4 * 2 * L], I16, kind="ExternalInput").ap())
    v_d = nc.dram_tensor("v", [128, B * 8 * D], BF16, kind="ExternalInput").ap()
    wq_d = nc.dram_tensor("wq", [128, 2 * D], F32, kind="ExternalInput").ap()
    wk_d = nc.dram_tensor("wk", [128, 2 * D], F32, kind="ExternalInput").ap()
    wv_d = nc.dram_tensor("wv", [128, 2 * D], BF16, kind="ExternalInput").ap()
    blk3_d = nc.dram_tensor("blk3", [8, 8, 8], F32, kind="ExternalInput").ap()
    wp_d = nc.dram_tensor("wp", [128, 2 * NSH], BF16, kind="ExternalInput").ap()
    out_d = nc.dram_tensor("out", [B, NSH], BF16, kind="ExternalOutput").ap()

    with tile.TileContext(nc) as tc:
        with (
            tc.tile_pool(name="cst", bufs=1) as cst,
            tc.tile_pool(name="qkp", bufs=6) as qkp,
            tc.tile_pool(name="work", bufs=2) as work,
            tc.tile_pool(name="wpp", bufs=12) as wpp,
            tc.tile_pool(name="outp", bufs=2) as outp,
            tc.tile_pool(name="ps_w", bufs=1, space="PSUM") as ps_w,
            tc.tile_pool(name="ps_r", bufs=1, space="PSUM") as ps_r,
            tc.tile_pool(name="ps_tp", bufs=2, space="PSUM") as ps_tp,
            tc.tile_pool(name="ps_out", bufs=3, space="PSUM") as ps_out,
        ):
            # ---------------- constants + PE warm-up ----------------
            one1 = cst.tile([1, 1], F32)
            nc.vector.memset(one1[:, :], 1.0)
            ident8 = cst.tile([8, 8], F32)
            make_identity(nc, ident8[:, :])

            junk = cst.tile([128, 512], BF16)
            nc.vector.memset(junk[:, :], 0.01)
            wps = ps_w.tile([128, 512], F32, tag="warm")
            for i in range(5):
                nc.tensor.matmul(wps[:, :], junk[:, 0:128], junk[:, :],
                                 start=(i == 0), stop=(i == 4))
            junk2 = cst.tile([128, 512], F32)
            nc.vector.tensor_copy(junk2[:, :], wps[:, :])

            # ---------------- input DMAs ----------------
            # sync queue: wq, wk, q/k batches 0-3, then the Wp stream.
            # gpsimd queue: wv, blk3, q/k batches 4-7, then v.
            wq_sb = cst.tile([128, 2, D], F32)
            nc.sync.dma_start(wq_sb[:, :, :], wq_d.rearrange("p (c d) -> p c d", c=2))
            wk_sb = cst.tile([128, 2, D], F32)
            nc.sync.dma_start(wk_sb[:, :, :], wk_d.rearrange("p (c d) -> p c d", c=2))
            wv_sb = cst.tile([128, 2, D], BF16)
            nc.gpsimd.dma_start(wv_sb[:, :, :], wv_d.rearrange("p (c d) -> p c d", c=2))
            blk3 = cst.tile([8, 8, 8], F32)
            nc.gpsimd.dma_start(blk3[:, :, :], blk3_d)

            qk_sb = {}
            for b in range(B):
                g, o = (0, b) if b < 4 else (1, b - 4)
                eng = nc.sync if b < 4 else nc.gpsimd
                qt = qkp.tile([128, 2, L], I16, tag="qk")
                eng.dma_start(qt[:, :, :],
                              qts[g][:, 2 * L * o:2 * L * (o + 1)]
                              .rearrange("p (c l) -> p c l", c=2))
                kt = qkp.tile([128, 2, L], I16, tag="qk")
                eng.dma_start(kt[:, :, :],
                              kts[g][:, 2 * L * o:2 * L * (o + 1)]
                              .rearrange("p (c l) -> p c l", c=2))
                qk_sb[b] = (qt, kt)
            # v rides the qk ring (same 4KB tile footprint, consumed right
            # after the corr loop) so SBUF affords a deeper Wp prefetch ring
            v_tiles = []
            for b in range(B):
                vt = qkp.tile([128, 8, D], BF16, tag="qk")
                nc.gpsimd.dma_start(
                    vt[:, :, :],
                    v_d[:, 8 * D * b:8 * D * (b + 1)].rearrange("p (t d) -> p t d", t=8))
                v_tiles.append(vt)

            # Wp shard: 16 x 1MB bf16 tiles, split across both queues behind
            # the q/k loads so the corr phase gets the full HBM bandwidth.
            wpts = []
            for nt in range(N_TILES):
                wpt = wpp.tile([128, 2, TILE_N], BF16, tag="wp")
                eng = nc.sync if nt % 2 == 0 else nc.gpsimd
                eng.dma_start(
                    wpt[:, :, :],
                    wp_d[:, 2 * TILE_N * nt:2 * TILE_N * (nt + 1)]
                    .rearrange("p (c n) -> p c n", c=2))
                wpts.append(wpt)

            # ---------------- head sums of Wq/Wk ----------------
            wqs = cst.tile([128, 2, 8], F32)
            nc.vector.reduce_sum(out=wqs[:, :, :],
                                 in_=wq_sb[:, :, :].rearrange("p c (h z) -> p c h z", z=DK),
                                 axis=mybir.AxisListType.X)
            wks = cst.tile([128, 2, 8], F32)
            nc.vector.reduce_sum(out=wks[:, :, :],
                                 in_=wk_sb[:, :, :].rearrange("p c (h z) -> p c h z", z=DK),
                                 axis=mybir.AxisListType.X)

            # ---------------- per-batch q/k projections -> corr rows ----------------
            # The blk3 h-reduction for batch b-1 is emitted between batch b's
            # sq/sk matmuls and batch b+1's, so the in-order PE never stalls
            # waiting for the DVE prod of the batch it just projected.
            ps_rr = ps_r.tile([8, L], F32, tag="r")
            prods = {}

            def emit_r(bb):
                for half in range(2):
                    sl = slice(512 * half, 512 * (half + 1))
                    nc.tensor.matmul(ps_rr[:, sl], blk3[:, bb, :], prods[bb][:, sl],
                                     start=(bb == 0), stop=(bb == B - 1))

            for b in range(B):
                qt, kt = qk_sb[b]
                # exact int16 -> fp32 cast; the fixed-point scale is folded
                # into Wq/Wk on the host
                qf = work.tile([128, 2, L], F32, tag="qf")
                nc.vector.tensor_copy(qf[:, :, :], qt[:, :, :])
                kf = work.tile([128, 2, L], F32, tag="kf")
                nc.vector.tensor_copy(kf[:, :, :], kt[:, :, :])
                xsT = {}
                for (t_sb, w_sum, nm) in ((qf, wqs, "q"), (kf, wks, "k")):
                    xs = work.tile([8, L], F32, tag=f"{nm}sT")
                    for half in range(2):
                        sl = slice(512 * half, 512 * (half + 1))
                        ps_x = ps_out.tile([8, 512], F32, tag="po")
                        nc.tensor.matmul(ps_x[:, :], w_sum[:, 0, :], t_sb[:, 0, sl], start=True, stop=False)
                        nc.tensor.matmul(ps_x[:, :], w_sum[:, 1, :], t_sb[:, 1, sl], start=False, stop=True)
                        if nm == "q":
                            nc.scalar.copy(xs[:, sl], ps_x[:, :])
                        else:
                            nc.vector.tensor_copy(xs[:, sl], ps_x[:, :])
                    xsT[nm] = xs
                prod = work.tile([8, L], F32, tag="prod")
                nc.vector.tensor_mul(prod[:, :], xsT["q"][:, :], xsT["k"][:, :])
                prods[b] = prod
                if b >= 1:
                    emit_r(b - 1)
            emit_r(B - 1)

            # ---------------- corr, top-6, softmax, select ----------------
            r_sb = cst.tile([8, L], F32)
            nc.vector.tensor_copy(r_sb[:, :], ps_rr[:, :])
            topv = cst.tile([8, 8], F32)
            nc.vector.max(topv[:, :], r_sb[:, :])
            negm0 = cst.tile([8, 1], F32)
            nc.vector.tensor_scalar_mul(negm0[:, :], topv[:, 0:1], -1.0)
            e_sb = cst.tile([8, K_TOP], F32)
            nc.scalar.activation(e_sb[:, :], topv[:, 0:K_TOP],
                                 mybir.ActivationFunctionType.Exp,
                                 bias=negm0[:, 0:1], scale=1.0)
            z_sb = cst.tile([8, 1], F32)
            nc.vector.reduce_sum(out=z_sb[:, :], in_=e_sb[:, :], axis=mybir.AxisListType.X)
            zinv = cst.tile([8, 1], F32)
            nc.vector.reciprocal(zinv[:, :], z_sb[:, :])
            w_sb = cst.tile([8, K_TOP], F32)
            nc.vector.tensor_scalar_mul(w_sb[:, :], e_sb[:, :], zinv[:, 0:1])

            # selu[b, l] = sum_j w_j * (r[b, l] == topv[b, j])
            selu = cst.tile([8, L], F32)
            ohw = cst.tile([8, L], F32)
            for j in range(K_TOP):
                dst = selu if j == 0 else ohw
                nc.vector.tensor_scalar(
                    out=dst[:, :], in0=r_sb[:, :],
                    scalar1=topv[:, j:j + 1], scalar2=w_sb[:, j:j + 1],
                    op0=mybir.AluOpType.is_equal, op1=mybir.AluOpType.mult)
                if j > 0:
                    nc.vector.tensor_add(selu[:, :], selu[:, :], ohw[:, :])

            # transpose sel to [l_low(128), 8t+b] layout (bf16)
            selT = cst.tile([128, 64], BF16)
            for t in range(8):
                tp8 = ps_tp.tile([128, 8], F32, tag="tp")
                nc.tensor.transpose(tp8[:, :], selu[0:8, 128 * t:128 * (t + 1)], ident8[:, :])
                nc.vector.tensor_copy(selT[:, 8 * t:8 * (t + 1)], tp8[:, :])

            # vbarT[e, b] = sum_l v[b, l, e] * sel[b, l]
            vbarT = cst.tile([128, 16], BF16)
            for b in range(B):
                for e in range(2):
                    pv = ps_tp.tile([128, 1], F32, tag="tp")
                    for t in range(8):
                        nc.tensor.matmul(pv[:, :],
                                         v_tiles[b][:, t, 128 * e:128 * (e + 1)],
                                         selT[:, 8 * t + b:8 * t + b + 1],
                                         start=(t == 0), stop=(t == 7))
                    nc.vector.tensor_copy(vbarT[:, 8 * e + b:8 * e + b + 1], pv[:, :])

            # aggT[d', b] = sum_e Wv[e, d'] * vbarT[e, b]   (bf16 out)
            aggt_bf = cst.tile([128, 16], BF16)
            for m in range(2):
                ps_a = ps_tp.tile([128, 8], F32, tag="tp")
                nc.tensor.matmul(ps_a[:, :], wv_sb[:, 0, 128 * m:128 * (m + 1)],
                                 vbarT[:, 0:8], start=True, stop=False)
                nc.tensor.matmul(ps_a[:, :], wv_sb[:, 1, 128 * m:128 * (m + 1)],
                                 vbarT[:, 8:16], start=False, stop=True)
                nc.vector.tensor_copy(aggt_bf[:, 8 * m:8 * (m + 1)], ps_a[:, :])

            # ---------------- big output projection (column shard) ----------------
            for nt in range(N_TILES):
                wpt = wpts[nt]
                o_sb = outp.tile([8, TILE_N], BF16)
                for s in range(SUBS):
                    ssl = slice(512 * s, 512 * (s + 1))
                    ps = ps_out.tile([8, 512], F32, tag="po")
                    nc.tensor.matmul(ps[:, :], aggt_bf[:, 0:8], wpt[:, 0, ssl], start=True, stop=False)
                    nc.tensor.matmul(ps[:, :], aggt_bf[:, 8:16], wpt[:, 1, ssl], start=False, stop=True)
                    if s % 2 == 0:
                        nc.scalar.copy(o_sb[:, ssl], ps[:, :])
                    else:
                        nc.vector.tensor_copy(o_sb[:, ssl], ps[:, :])
                nc.scalar.dma_start(out_d[:, TILE_N * nt:TILE_N * (nt + 1)], o_sb[:, :])

    nc.finalize()
    return nc


def _build_nc(with_bias, mode):
    nc = bacc.Bacc("TRN2", target_bir_lowering=False, debug=False, num_devices=N_CORES)

    qt_d = nc.dram_tensor("qt", [128, 2 * L], F32, kind="ExternalInput").ap()
    kt_d = nc.dram_tensor("kt", [128, 2 * L], F32, kind="ExternalInput").ap()
    v_d = nc.dram_tensor("v", [128, 8 * D], BF16, kind="ExternalInput").ap()
    wq_d = nc.dram_tensor("wq", [128, 2 * D], F32, kind="ExternalInput").ap()
    wk_d = nc.dram_tensor("wk", [128, 2 * D], F32, kind="ExternalInput").ap()
    wv_d = nc.dram_tensor("wv", [128, 2 * D], BF16, kind="ExternalInput").ap()
    wp_d = nc.dram_tensor("wp", [128, 2 * NSH], BF16, kind="ExternalInput").ap()
    if with_bias:
        bq_d = nc.dram_tensor("bq", [1, D], F32, kind="ExternalInput").ap()
        bk_d = nc.dram_tensor("bk", [1, D], F32, kind="ExternalInput").ap()
        bv_d = nc.dram_tensor("bv", [1, D], F32, kind="ExternalInput").ap()
        bp_d = nc.dram_tensor("bp", [N_TILES, TILE_N], F32, kind="ExternalInput").ap()
    out_d = nc.dram_tensor("out", [B, NSH], BF16, kind="ExternalOutput").ap()

    with tile.TileContext(nc) as tc:
        with (
            tc.tile_pool(name="cst", bufs=1) as cst,
            tc.tile_pool(name="wpp", bufs=WP_BUFS) as wpp,
            tc.tile_pool(name="outp", bufs=3) as outp,
            tc.tile_pool(name="bpp", bufs=2) as bpp,
            tc.tile_pool(name="dr", bufs=1, space="DRAM") as dr,
            tc.tile_pool(name="ps_w", bufs=1, space="PSUM") as ps_w,
            tc.tile_pool(name="ps_r", bufs=1, space="PSUM") as ps_r,
            tc.tile_pool(name="ps_tp", bufs=2, space="PSUM") as ps_tp,
            tc.tile_pool(name="ps_out", bufs=3, space="PSUM") as ps_out,
        ):
            # ---------------- constants + PE warm-up ----------------
            one1 = cst.tile([1, 1], F32)
            nc.vector.memset(one1[:, :], 1.0)
            ones128 = cst.tile([128, 1], F32)
            nc.vector.memset(ones128[:, :], 1.0)
            sones = cst.tile([8, 1], F32)
            nc.vector.memset(sones[:, :], SCALE)
            ident8 = cst.tile([8, 8], F32)
            make_identity(nc, ident8[:, :])

            junk = cst.tile([128, 512], BF16)
            nc.vector.memset(junk[:, :], 0.01)
            wps = ps_w.tile([128, 512], F32, tag="warm")
            for i in range(5):
                nc.tensor.matmul(wps[:, :], junk[:, 0:128], junk[:, :],
                                 start=(i == 0), stop=(i == 4))
            junk2 = cst.tile([128, 512], F32)
            nc.vector.tensor_copy(junk2[:, :], wps[:, :])

            # ---------------- input DMAs ----------------
            # latency-critical loads lead the sync queue in dependency order;
            # the 16MB Wp shard streams behind them.  Bulk-but-late tensors
            # (v, wv, biases) ride the gpsimd queue.
            wq_sb = cst.tile([128, 2, D], F32)
            nc.sync.dma_start(wq_sb[:, :, :], wq_d.rearrange("p (c d) -> p c d", c=2))
            wk_sb = cst.tile([128, 2, D], F32)
            nc.sync.dma_start(wk_sb[:, :, :], wk_d.rearrange("p (c d) -> p c d", c=2))
            qt_sb = cst.tile([128, 2, L], F32)
            nc.sync.dma_start(qt_sb[:, :, :], qt_d.rearrange("p (c l) -> p c l", c=2))
            kt_sb = cst.tile([128, 2, L], F32)
            nc.sync.dma_start(kt_sb[:, :, :], kt_d.rearrange("p (c l) -> p c l", c=2))

            v_sb = cst.tile([128, 8, D], BF16)
            nc.gpsimd.dma_start(v_sb[:, :, :], v_d.rearrange("p (t d) -> p t d", t=8))
            wv_sb = cst.tile([128, 2, D], BF16)
            nc.gpsimd.dma_start(wv_sb[:, :, :], wv_d.rearrange("p (c d) -> p c d", c=2))
            if with_bias:
                bq_sb = cst.tile([1, D], F32)
                nc.gpsimd.dma_start(bq_sb[:, :], bq_d)
                bk_sb = cst.tile([1, D], F32)
                nc.gpsimd.dma_start(bk_sb[:, :], bk_d)
                bv_sb = cst.tile([1, D], F32)
                nc.gpsimd.dma_start(bv_sb[:, :], bv_d)
                bp_sb = cst.tile([N_TILES, TILE_N], F32)
                nc.gpsimd.dma_start(bp_sb[:, :], bp_d)

            # Wp shard: 16 x 1MB bf16 tiles streamed on the sync queue.
            wpts = []
            for nt in range(N_TILES):
                wpt = wpp.tile([128, 2, TILE_N], BF16, tag="wp")
                nc.sync.dma_start(
                    wpt[:, :, :],
                    wp_d[:, 2 * TILE_N * nt:2 * TILE_N * (nt + 1)]
                    .rearrange("p (c n) -> p c n", c=2))
                wpts.append(wpt)

            # ---------------- head sums of Wq/Wk ----------------
            wqs = cst.tile([128, 2, 8], F32)
            nc.vector.reduce_sum(out=wqs[:, :, :],
                                 in_=wq_sb[:, :, :].rearrange("p c (h z) -> p c h z", z=DK),
                                 axis=mybir.AxisListType.X)
            wks = cst.tile([128, 2, 8], F32)
            nc.vector.reduce_sum(out=wks[:, :, :],
                                 in_=wk_sb[:, :, :].rearrange("p c (h z) -> p c h z", z=DK),
                                 axis=mybir.AxisListType.X)
            if with_bias:
                bqs_row = cst.tile([1, 8], F32)
                nc.vector.reduce_sum(out=bqs_row[:, :],
                                     in_=bq_sb[:, :].rearrange("o (h z) -> o h z", z=DK),
                                     axis=mybir.AxisListType.X)
                bks_row = cst.tile([1, 8], F32)
                nc.vector.reduce_sum(out=bks_row[:, :],
                                     in_=bk_sb[:, :].rearrange("o (h z) -> o h z", z=DK),
                                     axis=mybir.AxisListType.X)

            # ---------------- q/k head-sum projections (fp32) ----------------
            # sq/sk land in PSUM; the scalar engine drains sq to SBUF (adding
            # the head bias in the general variant) while the PE moves on.
            ps_q = []
            ps_k = []
            for (t_sb, w_sum, ps_list) in ((qt_sb, wqs, ps_q), (kt_sb, wks, ps_k)):
                for half in range(2):
                    sl = slice(512 * half, 512 * (half + 1))
                    ps_x = ps_out.tile([8, 512], F32, tag="po")
                    nc.tensor.matmul(ps_x[:, :], w_sum[:, 0, :], t_sb[:, 0, sl], start=True, stop=False)
                    nc.tensor.matmul(ps_x[:, :], w_sum[:, 1, :], t_sb[:, 1, sl], start=False, stop=True)
                    ps_list.append(ps_x)

            if with_bias:
                bqs_ps = ps_tp.tile([8, 1], F32, tag="tp")
                nc.tensor.matmul(bqs_ps[:, :], bqs_row[:, :], one1[:, :], start=True, stop=True)
                bqs_vert = cst.tile([8, 1], F32)
                nc.vector.tensor_copy(bqs_vert[:, :], bqs_ps[:, :])
                bks_ps = ps_tp.tile([8, 1], F32, tag="tp")
                nc.tensor.matmul(bks_ps[:, :], bks_row[:, :], one1[:, :], start=True, stop=True)
                bks_vert = cst.tile([8, 1], F32)
                nc.vector.tensor_copy(bks_vert[:, :], bks_ps[:, :])

            sq_sb = cst.tile([8, L], F32)
            prod = cst.tile([8, L], F32)
            for half in range(2):
                sl = slice(512 * half, 512 * (half + 1))
                if with_bias:
                    nc.vector.tensor_scalar(
                        out=sq_sb[:, sl], in0=ps_q[half][:, :],
                        scalar1=bqs_vert[:, 0:1], scalar2=None,
                        op0=mybir.AluOpType.add)
                else:
                    nc.scalar.copy(sq_sb[:, sl], ps_q[half][:, :])
            for half in range(2):
                sl = slice(512 * half, 512 * (half + 1))
                if with_bias:
                    sk_sb = cst.tile([8, 512], F32, tag=f"sk{half}")
                    nc.vector.tensor_scalar(
                        out=sk_sb[:, :], in0=ps_k[half][:, :],
                        scalar1=bks_vert[:, 0:1], scalar2=None,
                        op0=mybir.AluOpType.add)
                    nc.vector.tensor_mul(prod[:, sl], sq_sb[:, sl], sk_sb[:, :])
                else:
                    # DVE reads the sk chunk straight from PSUM
                    nc.vector.tensor_mul(prod[:, sl], sq_sb[:, sl], ps_k[half][:, :])

            ps_rr = ps_r.tile([1, L], F32, tag="r")
            for half in range(2):
                sl = slice(512 * half, 512 * (half + 1))
                nc.tensor.matmul(ps_rr[:, sl], sones[:, :], prod[:, sl], start=True, stop=True)
            r_row = cst.tile([1, L], F32)
            nc.scalar.copy(r_row[:, :], ps_rr[:, :])

            # ---------------- top-6 + masked softmax weights ----------------
            topv = cst.tile([1, 8], F32)
            nc.vector.max(topv[:, :], r_row[:, :])
            tvb = cst.tile([128, 8], F32)
            nc.gpsimd.partition_broadcast(tvb[:, :], topv[:, :])
            ntv0 = cst.tile([128, 1], F32)
            nc.vector.tensor_scalar_mul(ntv0[:, :], tvb[:, 0:1], -1.0)

            # rT[p, c] = r[128 c + p] via K=1 transpose-matmuls
            ps_rT = ps_tp.tile([128, 8], F32, tag="tp")
            for c in range(8):
                nc.tensor.matmul(ps_rT[:, c:c + 1], r_row[0:1, 128 * c:128 * (c + 1)],
                                 one1[:, :], start=True, stop=True)
            rT = cst.tile([128, 8], F32)
            nc.vector.tensor_copy(rT[:, :], ps_rT[:, :])

            eT = cst.tile([128, 8], F32)
            nc.scalar.activation(eT[:, :], rT[:, :],
                                 mybir.ActivationFunctionType.Exp,
                                 bias=ntv0[:, 0:1], scale=1.0)
            maskT = cst.tile([128, 8], F32)
            nc.vector.tensor_scalar(
                out=maskT[:, :], in0=rT[:, :],
                scalar1=tvb[:, K_TOP - 1:K_TOP], scalar2=None,
                op0=mybir.AluOpType.is_ge)
            ze = cst.tile([128, 8], F32)
            nc.vector.tensor_mul(ze[:, :], eT[:, :], maskT[:, :])
            ze_bf = cst.tile([128, 8], BF16)
            nc.vector.tensor_copy(ze_bf[:, :], ze[:, :])
            s1 = cst.tile([128, 1], F32)
            nc.vector.reduce_sum(out=s1[:, :], in_=ze[:, :], axis=mybir.AxisListType.X)
            Zps = ps_tp.tile([1, 1], F32, tag="tp")
            nc.tensor.matmul(Zps[:, :], s1[:, 0:1], ones128[:, :], start=True, stop=True)
            Zsb = cst.tile([1, 1], F32)
            nc.vector.tensor_copy(Zsb[:, :], Zps[:, :])
            zinv = cst.tile([1, 1], F32)
            nc.vector.reciprocal(zinv[:, :], Zsb[:, :])
            zvb = cst.tile([128, 1], F32)
            nc.gpsimd.partition_broadcast(zvb[:, :], zinv[:, :])

            # ---------------- weighted value aggregation (PE) ----------------
            # vbar[e] = (1/Z) sum_l v[l, e] * ze[l]; softmax normalization is
            # folded into the PSUM drain so the PE never waits on zvb.
            vbarT = cst.tile([128, 2], BF16)
            for m in range(2):
                pv = ps_tp.tile([128, 1], F32, tag="tp")
                for t in range(8):
                    nc.tensor.matmul(pv[:, :], v_sb[:, t, 128 * m:128 * (m + 1)],
                                     ze_bf[:, t:t + 1], start=(t == 0), stop=(t == 7))
                nc.vector.tensor_scalar_mul(vbarT[:, m:m + 1], pv[:, :], zvb[:, 0:1])

            # agg[d'] = sum_e Wv[e, d'] vbar[e] (+ bv)  -> [128, 2] (d' halves)
            aggT2 = cst.tile([128, 2], F32)
            for m in range(2):
                pa = ps_tp.tile([128, 1], F32, tag="tp")
                nc.tensor.matmul(pa[:, :], wv_sb[:, 0, 128 * m:128 * (m + 1)],
                                 vbarT[:, 0:1], start=True, stop=(False if with_bias else False))
                nc.tensor.matmul(pa[:, :], wv_sb[:, 1, 128 * m:128 * (m + 1)],
                                 vbarT[:, 1:2], start=False, stop=not with_bias)
                if with_bias:
                    nc.tensor.matmul(pa[:, :], bv_sb[0:1, 128 * m:128 * (m + 1)],
                                     one1[:, :], start=False, stop=True)
                nc.vector.tensor_copy(aggT2[:, m:m + 1], pa[:, :])

            # ---------------- agg exchange -> aggt_bf [128, 16] ----------------
            aggt_bf = cst.tile([128, 16], BF16)
            if mode == "rdma":
                # Direct peer-DMA all-gather: every core broadcasts its
                # aggT2 [128, 2] to slot k of peer (my_tpb XOR k).  Receiver
                # j's slot k therefore holds batch j^k; the host undoes the
                # XOR row permutation for free.  Each live dest bumps the
                # receiver's rem_sem by 2 -> 16 when all 8 rows landed.
                # The rem_sem wait is attached to the rx-consumer copies
                # AFTER Tile scheduling (see below) so the single-core
                # scheduling sim never sees a remotely-satisfied wait.
                rx = cst.tile([128, 8, 2], F32)
                rem_sem = nc.alloc_semaphore("agg_rx_sem")
                loc_sem = nc.alloc_semaphore("agg_tx_sem")
                for k in range(N_CORES):
                    rdests = [None] * N_CORES
                    rdests[k] = (0, k)
                    nc.gpsimd.remote_dma_broadcast(
                        rx[:, k, :], aggT2[:, :], rem_sem, loc_sem, rdests=rdests)
                nc.gpsimd.trigger_dma(count=N_CORES)
                # The NOP is the wait carrier: it gets the rem_sem >= 16
                # condition attached post-scheduling, stalling the in-order
                # DVE stream until every peer row has landed.
                wait_nop = nc.vector.nop(nofuse=True, hint="rx_wait")
                # aggt_bf[:, m*8 + k] = rx[:, k, m]
                for m in range(2):
                    nc.vector.tensor_copy(aggt_bf[:, 8 * m:8 * (m + 1)], rx[:, :, m])
                nc._rdma_rx_consumers = [wait_nop]
                nc._rdma_rem_sem = rem_sem
            else:
                agg_in = dr.tile([1, D], F32)
                nc.gpsimd.dma_start(
                    agg_in[:, :].rearrange("o (m e) -> (o e) m", e=128), aggT2[:, :])
                agg_out = dr.tile([B, D], F32)
                nc.gpsimd.collective_compute(
                    "AllGather", mybir.AluOpType.bypass,
                    replica_groups=[list(range(N_CORES))],
                    ins=[agg_in[:, :].opt()], outs=[agg_out[:, :].opt()])
                aggf = cst.tile([8, D], F32)
                nc.gpsimd.dma_start(aggf[:, :], agg_out[:, :])
                for m in range(2):
                    pt = ps_tp.tile([128, 8], F32, tag="tp")
                    nc.tensor.transpose(pt[:, :], aggf[0:8, 128 * m:128 * (m + 1)], ident8[:, :])
                    nc.vector.tensor_copy(aggt_bf[:, 8 * m:8 * (m + 1)], pt[:, :])

            # ---------------- big output projection (column shard) ----------------
            for nt in range(N_TILES):
                wpt = wpts[nt]
                if with_bias:
                    bp_rep = bpp.tile([8, TILE_N], F32, tag="bprep")
                    nc.gpsimd.partition_broadcast(bp_rep[:, :], bp_sb[nt:nt + 1, :])
                o_sb = outp.tile([8, TILE_N], BF16)
                for s in range(SUBS):
                    ssl = slice(512 * s, 512 * (s + 1))
                    ps = ps_out.tile([8, 512], F32, tag="po")
                    nc.tensor.matmul(ps[:, :], aggt_bf[:, 0:8], wpt[:, 0, ssl], start=True, stop=False)
                    nc.tensor.matmul(ps[:, :], aggt_bf[:, 8:16], wpt[:, 1, ssl], start=False, stop=True)
                    if s % 2 == 0:
                        nc.scalar.copy(o_sb[:, ssl], ps[:, :])
                    else:
                        nc.vector.tensor_copy(o_sb[:, ssl], ps[:, :])
                if with_bias:
                    nc.vector.tensor_add(o_sb[:, :], o_sb[:, :], bp_rep[:, :])
                nc.scalar.dma_start(out_d[:, TILE_N * nt:TILE_N * (nt + 1)], o_sb[:, :])

    if mode == "rdma":
        # Tile scheduling ran at context exit; only now attach the
        # remotely-satisfied wait so the rx copies stall until all 8 peer
        # rows have physically landed.
        for ci in nc._rdma_rx_consumers:
            ci._wait_ge(nc._rdma_rem_sem, 16)
    nc.finalize()
    return nc


def _get_nc(mode, with_bias):
    key = (mode, with_bias)
    if key not in _CACHE:
        _CACHE[key] = _build_nc_rep() if mode == "rep" else _build_nc(with_bias, mode)
    return _CACHE[key]


def kernel(queries, keys, values, Wq, bq, Wk, bk, Wv, bv, Wp, bp):
    q3 = np.asarray(queries, np.float32).reshape(B, L, D)
    k3 = np.asarray(keys, np.float32).reshape(B, L, D)
    v3 = np.asarray(values, np.float32).reshape(B, L, D)
    Wq = np.asarray(Wq, np.float32)
    Wk = np.asarray(Wk, np.float32)
    Wv = np.asarray(Wv, np.float32)
    bq = np.asarray(bq, np.float32).reshape(1, D)
    bk = np.asarray(bk, np.float32).reshape(1, D)
    bv = np.asarray(bv, np.float32).reshape(1, D)
    Wp = np.asarray(Wp, np.float32)
    bp = np.asarray(bp, np.float32).reshape(-1)

    with_bias = bool(bq.any() or bk.any() or bv.any() or bp.any())
    # the rep program hardcodes zero biases; nonzero biases take the general
    # data-parallel + AllGather variant
    mode = MODE if not (MODE == "rep" and with_bias) else "cc"
    nc = _get_nc(mode, with_bias)

    if mode == "rep":
        # q/k ship as int16 fixed point (absolute step ~1.6e-4 keeps the
        # top-k corr margins with >4x headroom); the dequant scale is folded
        # into Wq/Wk so the device only does an exact int16->fp32 cast.
        sq_s = np.float32(32766.0 / max(np.abs(q3).max(), 1e-30))
        sk_s = np.float32(32766.0 / max(np.abs(k3).max(), 1e-30))
        wq_h = np.ascontiguousarray(
            (Wq / sq_s).reshape(2, 128, D).transpose(1, 0, 2).reshape(128, 2 * D))
        wk_h = np.ascontiguousarray(
            (Wk / sk_s).reshape(2, 128, D).transpose(1, 0, 2).reshape(128, 2 * D))
        wv_h = np.ascontiguousarray(
            Wv.reshape(2, 128, D).transpose(1, 0, 2).reshape(128, 2 * D)).astype(NPBF16)
        # [p, b, c, l] with d = 128 c + p, batch groups of 4
        qk_all = []
        for x3, s in ((q3, sq_s), (k3, sk_s)):
            xi = np.clip(np.rint(x3 * s), -32767, 32767).astype(np.int16)
            xt = xi.transpose(0, 2, 1).reshape(B, 2, 128, L).transpose(2, 0, 1, 3)
            qk_all.append(np.ascontiguousarray(xt.reshape(128, B * 2 * L)))
        qt_h, kt_h = qk_all
        v_h = np.ascontiguousarray(
            v3.reshape(B, 8, 128, D).transpose(2, 0, 1, 3).reshape(128, B * 8 * D)
        ).astype(NPBF16)
        blk3_h = np.zeros((8, 8, 8), np.float32)
        for b in range(B):
            blk3_h[:, b, b] = SCALE
        wp_bf = Wp.astype(NPBF16)
        in_maps = []
        for i in range(N_CORES):
            cols = slice(NSH * i, NSH * (i + 1))
            wp_h = np.ascontiguousarray(
                wp_bf[:, cols].reshape(2, 128, N_TILES, TILE_N)
                .transpose(1, 2, 0, 3).reshape(128, 2 * NSH))
            in_maps.append({
                "qt0": qt_h[:, :8 * L], "qt1": qt_h[:, 8 * L:],
                "kt0": kt_h[:, :8 * L], "kt1": kt_h[:, 8 * L:],
                "v": v_h, "wq": wq_h, "wk": wk_h, "wv": wv_h,
                "blk3": blk3_h, "wp": wp_h,
            })
        res = run_bass_kernel_spmd(nc, in_maps, core_ids=list(range(N_CORES)), trace=TRACE)
        global LAST_RESULT
        LAST_RESULT = res
        out = np.concatenate(
            [np.asarray(res.results[i]["out"]) for i in range(N_CORES)], axis=1)
        return out.astype(np.float32).reshape(B, L, D)

    # shared (replicated) weight layouts
    wq_h = np.ascontiguousarray(
        Wq.reshape(2, 128, D).transpose(1, 0, 2).reshape(128, 2 * D))
    wk_h = np.ascontiguousarray(
        Wk.reshape(2, 128, D).transpose(1, 0, 2).reshape(128, 2 * D))
    wv_h = np.ascontiguousarray(
        Wv.reshape(2, 128, D).transpose(1, 0, 2).reshape(128, 2 * D)).astype(NPBF16)

    wp_bf = Wp.astype(NPBF16)

    in_maps = []
    for i in range(N_CORES):
        cols = slice(NSH * i, NSH * (i + 1))
        # per-batch transposed q/k: [p, c, l] with d = 128 c + p
        qt_h = np.ascontiguousarray(
            q3[i].T.reshape(2, 128, L).transpose(1, 0, 2).reshape(128, 2 * L))
        kt_h = np.ascontiguousarray(
            k3[i].T.reshape(2, 128, L).transpose(1, 0, 2).reshape(128, 2 * L))
        # v in [p, t, d] with l = 128 t + p
        v_h = np.ascontiguousarray(
            v3[i].reshape(8, 128, D).transpose(1, 0, 2).reshape(128, 8 * D)).astype(NPBF16)
        # Wp shard in [p, nt, c, j] with d = 128 c + p, col = 2048 nt + j
        wp_h = np.ascontiguousarray(
            wp_bf[:, cols].reshape(2, 128, N_TILES, TILE_N)
            .transpose(1, 2, 0, 3).reshape(128, 2 * NSH))
        m = {
            "qt": qt_h, "kt": kt_h, "v": v_h,
            "wq": wq_h, "wk": wk_h, "wv": wv_h,
            "wp": wp_h,
        }
        if with_bias:
            m["bq"] = bq
            m["bk"] = bk
            m["bv"] = bv
            m["bp"] = np.ascontiguousarray(bp[cols]).reshape(N_TILES, TILE_N)
        in_maps.append(m)

    res = run_bass_kernel_spmd(nc, in_maps, core_ids=list(range(N_CORES)), trace=TRACE)
    LAST_RESULT = res
    out = np.empty((B, L * D), np.float32)
    for j in range(N_CORES):
        o = np.asarray(res.results[j]["out"]).astype(np.float32)
        if mode == "rdma":
            # core j's row r holds batch j ^ r (XOR-relative exchange slots)
            for r in range(B):
                out[j ^ r, NSH * j:NSH * (j + 1)] = o[r]
        else:
            out[:, NSH * j:NSH * (j + 1)] = o
    return out.reshape(B, L, D)
